# revision 5
# baseline (speedup 1.0000x reference)
"""AutoCorr2D Trainium2 Bass kernel.

Reference computation (per batch image, fp32):
  f   = conv3x3(x, W_ext, pad=1) + b_ext            # (16, 128, 128)
  corr[c,y,w,k] = f[c,y,w] * f_pad[c, y+ki-2, w+kj-2]   # 5x5 window, zero pad
  out[o,y,w]    = sum_{c,k} W_reg[o,c,ki,kj] * corr[c,y,w,k]

Sharding: pure data parallel — batch 8 -> one image per NeuronCore, weights
replicated. Each core runs an identical NEFF (SPMD) on its shard.

Per-core pipeline (strips of 16 rows, slices of 4 rows = 512 spatial):
  1. conv: 18 bf16 matmuls per slice (9 taps x 2 cin-chunks), K=128, M=32
     (each tap's 16 output channels duplicated to fill a 32-wide col group),
     round-robined over 4 PE col-groups via tile_position so the four
     accumulation chains run concurrently -> PSUM F4 [(group,dup,c)=128, 512].
  2. one fp32r matmul with a constant 0.5-weighted replication matrix fuses
     the 4-group reduction AND the 8x channel replication:
     A[(k8,c), s] = f[c, s]  (PSUM, fp32)
  3. ACT evacuates A to a persistent padded bf16 buffer A_sb [128, 132, 132]
     (adding b_ext via the activation bias port; borders stay zero = padding).
  4. The 25 shifted window views B[(k,c), s] = f[c, s+delta_k] are built with
     plain SBUF->SBUF DMA copies out of A_sb (partition-group shifts cannot be
     expressed as engine APs; DMA crosses partitions freely).
  5. DVE bf16 products g = A ⊙ B on all 128 partitions (2x packed mode).
  6. regressor: 4 bf16 matmuls per slice, K=(c,k) packed into
     {128,128,64,80}-partition chunks, accumulated in PSUM [64, 512].
  7. evac + DMA to HBM.
"""
import os

os.environ.setdefault("JAX_PLATFORMS", "cpu")  # ignored when axon env is active

import numpy as np

import concourse.bass as bass
import concourse.tile as tile
import concourse.mybir as mybir
from concourse import bacc
from concourse.bass_utils import run_bass_kernel_spmd

F32 = mybir.dt.float32
F32R = mybir.dt.float32r
BF16 = mybir.dt.bfloat16
AFT = mybir.ActivationFunctionType

B, CIN, H, W = 8, 256, 128, 128
CC, COUT = 16, 64
NCH = CIN // 128          # 2 cin chunks
NSTRIP, RSTRIP = 8, 16    # strips of 16 rows
NSLICE = RSTRIP // 4      # 4-row (512 element) slices per strip

# rectangular (ki, kj) chunks of the 5x5 autocorr window, <=8 taps each so
# (tap, channel) packs into <=128 partitions
RECTS = [
    (0, 2, 0, 4),   # ki 0-1, kj 0-3  -> 8 taps, 128 partitions
    (2, 4, 0, 4),   # ki 2-3, kj 0-3  -> 8 taps, 128 partitions
    (0, 4, 4, 5),   # ki 0-3, kj 4    -> 4 taps,  64 partitions
    (4, 5, 0, 5),   # ki 4,   kj 0-4  -> 5 taps,  80 partitions
]
RECT_P = [(r[1] - r[0]) * (r[3] - r[2]) * CC for r in RECTS]


def _build_nc():
    nc = bacc.Bacc("TRN2", debug=False, target_bir_lowering=False, num_devices=8)
    x = nc.dram_tensor("x", (CIN, H, W), F32, kind="ExternalInput").ap()
    w_ext = nc.dram_tensor("W_ext", (CC, CIN, 3, 3), F32, kind="ExternalInput").ap()
    b_ext = nc.dram_tensor("b_ext", (CC,), F32, kind="ExternalInput").ap()
    w_reg = nc.dram_tensor("W_reg", (COUT, CC, 5, 5), F32, kind="ExternalInput").ap()
    out = nc.dram_tensor("out", (COUT, H, W), F32, kind="ExternalOutput").ap()

    # replication/reduction matrix: A[(k8,c), s] = sum_g 0.5 * F4[(g,u,c), s]
    blk = 0.5 * np.tile(np.eye(CC, dtype=np.float32), (1, 8))    # [16, 128]
    r_np = np.tile(blk, (8, 1))                                  # [128, 128]
    r_const = nc.inline_tensor(r_np, name="Rmat").ap()

    x_r = x.rearrange("(ch p) h w -> ch p h w", p=128)

    with tile.TileContext(nc) as tc:
        import contextlib
        with contextlib.ExitStack() as ctx:
            cpool = ctx.enter_context(tc.tile_pool(name="consts", bufs=1))
            apool = ctx.enter_context(tc.tile_pool(name="asb", bufs=1))
            xfp = ctx.enter_context(tc.tile_pool(name="xf", bufs=2))
            xbp = ctx.enter_context(tc.tile_pool(name="xb", bufs=2))
            f4sp = ctx.enter_context(tc.tile_pool(name="f4s", bufs=3))
            bpool = ctx.enter_context(tc.tile_pool(name="bwin", bufs=2))
            gpool = ctx.enter_context(tc.tile_pool(name="g", bufs=4))
            opool = ctx.enter_context(tc.tile_pool(name="osb", bufs=4))
            f4pp = ctx.enter_context(tc.tile_pool(name="f4p", bufs=2, space="PSUM"))
            app = ctx.enter_context(tc.tile_pool(name="ap", bufs=2, space="PSUM"))
            opp = ctx.enter_context(tc.tile_pool(name="op", bufs=3, space="PSUM"))

            # ---- constants / weights staging ----
            # conv weights: col = ((di*3+dj)*2 + ch)*32 + u*16 + c, u = dup
            wc_f = cpool.tile([128, 576], F32)
            wc_src = w_ext.rearrange("c (ch p) i j -> i j ch p c", p=128)
            for di in range(3):
                for dj in range(3):
                    for ch in range(2):
                        t32 = ((di * 3 + dj) * 2 + ch) * 32
                        for u in range(2):
                            nc.sync.dma_start(
                                wc_f[:, t32 + 16 * u:t32 + 16 * (u + 1)],
                                wc_src[di, dj, ch])
            wc = cpool.tile([128, 576], BF16)
            nc.vector.tensor_copy(wc[:], wc_f[:])

            # replication matrix -> fp32r
            r_f = cpool.tile([128, 128], F32)
            nc.sync.dma_start(r_f[:], r_const)
            r_r = cpool.tile([128, 128], F32R)
            nc.scalar.copy(r_r[:], r_f[:])

            # regressor weights, packed per rect chunk into 64-col blocks
            wr_f = cpool.tile([128, 4 * COUT], F32)
            nc.vector.memset(wr_f[:], 0.0)
            wr_src = w_reg.rearrange("o c i j -> i j c o")
            for j, (i0, i1, j0, j1) in enumerate(RECTS):
                idx = 0
                for ki in range(i0, i1):
                    for kj in range(j0, j1):
                        nc.sync.dma_start(
                            wr_f[CC * idx:CC * (idx + 1),
                                 j * COUT:(j + 1) * COUT],
                            wr_src[ki, kj])
                        idx += 1
            wr = cpool.tile([128, 4 * COUT], BF16)
            nc.vector.tensor_copy(wr[:], wr_f[:])

            # bias replicated to all (k8, c) partitions
            bias_t = cpool.tile([128, 1], F32)
            bias_src = b_ext.unsqueeze(0).broadcast_to((8, CC)).unsqueeze(-1)
            nc.sync.dma_start(bias_t[:], bias_src)

            # persistent padded replicated f buffer; borders = zero padding
            a_sb = apool.tile([128, H + 4, W + 4], BF16)
            nc.gpsimd.memset(a_sb[:, 0:2, :], 0.0)
            nc.gpsimd.memset(a_sb[:, H + 2:H + 4, :], 0.0)
            nc.gpsimd.memset(a_sb[:, 2:H + 2, 0:2], 0.0)
            nc.gpsimd.memset(a_sb[:, 2:H + 2, W + 2:W + 4], 0.0)

            # conv group assignment: 18 matmuls round-robin over 4 col groups
            pairs = [(di, dj, ch) for di in range(3) for dj in range(3)
                     for ch in range(NCH)]
            grp = {p: i % 4 for i, p in enumerate(pairs)}
            gfirst, glast = {}, {}
            for p in pairs:
                g = grp[p]
                if g not in gfirst:
                    gfirst[g] = p
                glast[g] = p

            def conv_strip(s):
                r0 = s * RSTRIP
                lo = max(r0 - 1, 0)
                hi = min(r0 + RSTRIP + 1, H)
                tlo, thi = lo - (r0 - 1), hi - (r0 - 1)
                xf = xfp.tile([128, NCH, RSTRIP + 2, W], F32, tag="xf")
                for ch in range(NCH):
                    nc.sync.dma_start(xf[:, ch, tlo:thi, :], x_r[ch, :, lo:hi, :])
                xb = xbp.tile([128, NCH, RSTRIP + 2, W + 4], BF16, tag="xb")
                # zero the w-borders (cols 1 and 130 are read as x[-1]/x[128])
                nc.gpsimd.memset(xb[:, :, :, 0:2], 0.0)
                nc.gpsimd.memset(xb[:, :, :, W + 2:W + 4], 0.0)
                if tlo > 0:
                    nc.gpsimd.memset(xb[:, :, 0:tlo, :], 0.0)
                if thi < RSTRIP + 2:
                    nc.gpsimd.memset(xb[:, :, thi:RSTRIP + 2, :], 0.0)
                for ch in range(NCH):
                    nc.gpsimd.tensor_copy(xb[:, ch, tlo:thi, 2:W + 2],
                                          xf[:, ch, tlo:thi, :])

                for q in range(NSLICE):
                    h0 = r0 + 4 * q
                    f4 = f4pp.tile([128, 4, W], F32, tag="f4p")
                    for (di, dj, ch) in pairs:
                        g = grp[(di, dj, ch)]
                        t32 = ((di * 3 + dj) * 2 + ch) * 32
                        nc.tensor.matmul(
                            f4[32 * g:32 * g + 32, :, :],
                            wc[:, t32:t32 + 32],
                            xb[:, ch, 4 * q + di:4 * q + di + 4,
                               1 + dj:1 + dj + W],
                            start=(gfirst[g] == (di, dj, ch)),
                            stop=(glast[g] == (di, dj, ch)),
                            tile_position=(0, 32 * g),
                            skip_group_check=True,
                        )
                    f4s = f4sp.tile([128, 4, W], F32R, tag="f4s")
                    nc.scalar.copy(f4s[:], f4[:])
                    apt = app.tile([128, 4, W], F32, tag="ap")
                    nc.tensor.matmul(apt[:], r_r[:], f4s[:], start=True, stop=True)
                    nc.scalar.activation(a_sb[:, h0 + 2:h0 + 6, 2:W + 2],
                                         apt[:], AFT.Identity, bias=bias_t[:])

            def autocorr_strip(s):
                r0 = s * RSTRIP
                bts = []
                for j, (i0, i1, j0, j1) in enumerate(RECTS):
                    bt = bpool.tile([128, RSTRIP, W], BF16, tag=f"b{j}")
                    idx = 0
                    for ki in range(i0, i1):
                        for kj in range(j0, j1):
                            nc.sync.dma_start(
                                bt[CC * idx:CC * (idx + 1), :, :],
                                a_sb[0:CC, r0 + ki:r0 + ki + RSTRIP,
                                     kj:kj + W])
                            idx += 1
                    bts.append(bt)
                for q in range(NSLICE):
                    h0 = r0 + 4 * q
                    op = opp.tile([COUT, 4, W], F32, tag="op")
                    for j in range(4):
                        pj = RECT_P[j]
                        gt = gpool.tile([128, 4, W], BF16, tag="g")
                        nc.vector.tensor_mul(
                            gt[0:pj, :, :],
                            a_sb[0:pj, h0 + 2:h0 + 6, 2:W + 2],
                            bts[j][0:pj, 4 * q:4 * q + 4, :])
                        nc.tensor.matmul(op[:], wr[0:pj, j * COUT:(j + 1) * COUT],
                                         gt[0:pj, :, :],
                                         start=(j == 0), stop=(j == 3))
                    ot = opool.tile([COUT, 4, W], F32, tag="osb")
                    if q % 2 == 0:
                        nc.vector.tensor_copy(ot[:], op[:])
                    else:
                        nc.scalar.copy(ot[:], op[:])
                    nc.scalar.dma_start(out[:, h0:h0 + 4, :], ot[:])

            for s in range(NSTRIP + 1):
                if s < NSTRIP:
                    conv_strip(s)
                if s >= 1:
                    autocorr_strip(s - 1)

    nc.compile()
    return nc


_NC = None


def _get_nc():
    global _NC
    if _NC is None:
        _NC = _build_nc()
    return _NC


def kernel(x, W_ext, b_ext, W_reg):
    x = np.ascontiguousarray(np.asarray(x, dtype=np.float32))
    W_ext = np.ascontiguousarray(np.asarray(W_ext, dtype=np.float32))
    b_ext = np.ascontiguousarray(np.asarray(b_ext, dtype=np.float32))
    W_reg = np.ascontiguousarray(np.asarray(W_reg, dtype=np.float32))

    nc = _get_nc()
    in_maps = [
        {"x": np.ascontiguousarray(x[c]), "W_ext": W_ext, "b_ext": b_ext,
         "W_reg": W_reg}
        for c in range(B)
    ]
    res = run_bass_kernel_spmd(nc, in_maps, core_ids=list(range(B)))
    return np.stack([res.results[c]["out"] for c in range(B)], axis=0)


if __name__ == "__main__":
    rng = np.random.default_rng(0)
    inputs = {
        "x": rng.standard_normal((B, CIN, H, W), dtype=np.float32),
        "W_ext": (rng.standard_normal((CC, CIN, 3, 3)) * 0.05).astype(np.float32),
        "b_ext": (rng.standard_normal((CC,)) * 0.05).astype(np.float32),
        "W_reg": (rng.standard_normal((COUT, CC, 5, 5)) * 0.05).astype(np.float32),
    }
    y = kernel(**inputs)
    print("out", y.shape, y.dtype, float(np.abs(y).max()))


# revision 32
# speedup vs baseline: 7006.2236x; 7006.2236x over previous
"""AutoCorr2D Trainium2 Bass kernel (v3).

Reference computation (per batch image, fp32):
  f   = conv3x3(x, W_ext, pad=1) + b_ext            # (16, 128, 128)
  corr[c,y,w,k] = f[c,y,w] * f_pad[c, y+ki-2, w+kj-2]   # 5x5 window, zero pad
  out[o,y,w]    = sum_{c,k} W_reg[o,c,ki,kj] * corr[c,y,w,k]

Sharding: pure data parallel — batch 8 -> one image per NeuronCore, weights
replicated; identical NEFF on every core (SPMD).

Structure per core:
  - x (host-cast bf16) resident in SBUF, padded; conv = 18 bf16 matmuls per
    4-row slice (9 taps x 2 cin chunks, K=128, M=32 with each tap's 16
    channels duplicated), round-robined over 4 PE column groups
    (tile_position) so four PSUM accumulation chains run concurrently.
  - one fp32r matmul against a constant 0.5-replication matrix fuses the
    4-group reduction with 8x channel replication; ACT evacuates (+bias,
    bf16 cast) into a persistent padded buffer A_sb [128, 132, 132].
  - the 25 shifted window copies B[(k,c),h,w] = f[c,h+ki-2,w+kj-2] are
    whole-image SBUF->SBUF DMAs out of A_sb, spread over both HWDGE rings.
  - products g = A ⊙ B on DVE (bf16 2x mode, 128 partitions); regressor =
    4 bf16 matmuls per slice with K=(tap,channel) packed {128,128,64,80},
    PSUM-accumulated; outputs stored once per 16-row strip via SWDGE.
"""
import os

os.environ.setdefault("JAX_PLATFORMS", "cpu")  # ignored when axon env is active

import numpy as np
import ml_dtypes

import concourse.bass as bass
import concourse.tile as tile
import concourse.mybir as mybir
from concourse import bacc
from concourse.bass_utils import run_bass_kernel_spmd

F32 = mybir.dt.float32
F32R = mybir.dt.float32r
BF16 = mybir.dt.bfloat16
AFT = mybir.ActivationFunctionType

B, CIN, H, W = 8, 256, 128, 128
CC, COUT = 16, 64
NCH = CIN // 128          # 2 cin chunks
NSTRIP, RSTRIP = 8, 16    # strips of 16 rows
NSLICE = RSTRIP // 4      # 4-row (512 element) slices per strip

# rectangular (ki, kj) chunks of the 5x5 autocorr window, <=8 taps each so
# (tap, channel) packs into <=128 partitions
RECTS = [
    (0, 2, 0, 4),   # ki 0-1, kj 0-3  -> 8 taps, 128 partitions
    (2, 4, 0, 4),   # ki 2-3, kj 0-3  -> 8 taps, 128 partitions
    (0, 4, 4, 5),   # ki 0-3, kj 4    -> 4 taps,  64 partitions
    (4, 5, 0, 5),   # ki 4,   kj 0-4  -> 5 taps,  80 partitions
]
RECT_P = [(r[1] - r[0]) * (r[3] - r[2]) * CC for r in RECTS]


def _build_nc(loop_n=None):
    """loop_n: wrap the whole body in a device-side For_i loop (timing
    harness only — measures per-execution HW time free of host overhead)."""
    nc = bacc.Bacc("TRN2", debug=False, target_bir_lowering=False, num_devices=8)
    # x arrives host-padded: rows -1..128, cols -2..129 of the image (zeros
    # outside) so strip loads are single contiguous runs and conv reads never
    # need border memsets.
    x = nc.dram_tensor("x", (CIN, H + 2, W + 4), BF16, kind="ExternalInput").ap()
    w_ext = nc.dram_tensor("W_ext", (CC, CIN, 3, 3), BF16, kind="ExternalInput").ap()
    b_ext = nc.dram_tensor("b_ext", (CC,), F32, kind="ExternalInput").ap()
    w_reg = nc.dram_tensor("W_reg", (COUT, CC, 5, 5), BF16, kind="ExternalInput").ap()
    out = nc.dram_tensor("out", (COUT, H, W), F32, kind="ExternalOutput").ap()

    # replication/reduction matrix: A[(k8,c), s] = sum_g 0.5 * F4[(g,u,c), s]
    blk = 0.5 * np.tile(np.eye(CC, dtype=np.float32), (1, 8))    # [16, 128]
    r_np = np.tile(blk, (8, 1))                                  # [128, 128]
    r_const = nc.inline_tensor(r_np, name="Rmat").ap()

    x_r = x.rearrange("(ch p) h w -> ch p h w", p=128)

    with tile.TileContext(nc) as tc:
        import contextlib
        with contextlib.ExitStack() as ctx:
            pools = {
                "cpool": ctx.enter_context(tc.tile_pool(name="consts", bufs=1)),
                "apool": ctx.enter_context(tc.tile_pool(name="asb", bufs=1)),
                "xpool": ctx.enter_context(tc.tile_pool(name="ximg", bufs=1)),
                "f4sp": ctx.enter_context(tc.tile_pool(name="f4s", bufs=3)),
                "bpool": ctx.enter_context(tc.tile_pool(name="bwin", bufs=1)),
                "gpool": ctx.enter_context(tc.tile_pool(name="g", bufs=4)),
                "opool": ctx.enter_context(tc.tile_pool(name="osb", bufs=2)),
                "f4pp": ctx.enter_context(
                    tc.tile_pool(name="f4p", bufs=2, space="PSUM")),
                "app": ctx.enter_context(
                    tc.tile_pool(name="ap", bufs=2, space="PSUM")),
                "opp": ctx.enter_context(
                    tc.tile_pool(name="op", bufs=3, space="PSUM")),
            }

            def whole_body():
                _emit_body(nc, tc, pools, x_r, w_ext, b_ext, w_reg, out,
                           r_const)

            if loop_n is not None:
                with tc.For_i(0, loop_n, 1,
                              hint_engines=tuple(mybir.ALL_ENGINES)):
                    whole_body()
            else:
                whole_body()

    nc.compile()
    return nc


def _emit_body(nc, tc, pools, x_r, w_ext, b_ext, w_reg, out, r_const):
    cpool, apool, xpool = pools["cpool"], pools["apool"], pools["xpool"]
    f4sp, bpool, gpool, opool = (pools["f4s" "p"], pools["bpool"],
                                 pools["gpool"], pools["opool"])
    f4pp, app, opp = pools["f4pp"], pools["app"], pools["opp"]

    phase = os.environ.get("KPHASE", "full")
    if phase == "empty":
        nc.scalar.nop()
        return

    # ---- setup: weights / constants ----
    wc = cpool.tile([128, 576], BF16)
    wc_src = w_ext.rearrange("c (ch p) i j -> i j ch p c", p=128)
    for di in range(3):
        for dj in range(3):
            for ch in range(NCH):
                t32 = ((di * 3 + dj) * 2 + ch) * 32
                nc.sync.dma_start(wc[:, t32:t32 + CC], wc_src[di, dj, ch])
    # duplicate each tap's 16 cols into the second 16 (one strided copy)
    wc_v = wc[:].rearrange("p (t u c) -> p t u c", t=18, u=2)
    nc.gpsimd.tensor_copy(wc_v[:, :, 1, :], wc_v[:, :, 0, :])

    r_f = cpool.tile([128, 128], F32)
    nc.sync.dma_start(r_f[:], r_const)
    r_r = cpool.tile([128, 128], F32R)
    nc.scalar.copy(r_r[:], r_f[:])

    wr = cpool.tile([128, 4 * COUT], BF16)
    nc.vector.memset(wr[:], 0.0)
    wr_src = w_reg.rearrange("o c i j -> i j c o")
    for j, (i0, i1, j0, j1) in enumerate(RECTS):
        idx = 0
        for ki in range(i0, i1):
            for kj in range(j0, j1):
                nc.scalar.dma_start(
                    wr[CC * idx:CC * (idx + 1), j * COUT:(j + 1) * COUT],
                    wr_src[ki, kj])
                idx += 1

    bias_t = cpool.tile([128, 1], F32)
    bias_src = b_ext.unsqueeze(0).broadcast_to((8, CC)).unsqueeze(-1)
    nc.sync.dma_start(bias_t[:], bias_src)

    # ---- x strips: 18 host-padded rows per 16-row strip; contiguous src
    # and dst -> one DMA descriptor per partition ----
    def load_x_strip(s):
        r0 = s * RSTRIP
        xb = xpool.tile([128, NCH, RSTRIP + 2, W + 4], BF16, tag="xb")
        for ch in range(NCH):
            eng = nc.sync if ch == 0 else nc.scalar
            eng.dma_start(xb[:, ch, :, :], x_r[ch, :, r0:r0 + RSTRIP + 2, :])
        return xb

    # ---- persistent padded replicated f buffer (flat alloc with a 4-elem
    # tail so the flat-wrap window copies below never read out of bounds) ----
    PW = W + 4
    a_flat = apool.tile([128, (H + 4) * PW + 4], BF16)
    a_sb = a_flat[:, 0:(H + 4) * PW].rearrange("p (h w) -> p h w", w=PW)
    nc.gpsimd.memset(a_flat[:, (H + 4) * PW:], 0.0)
    nc.gpsimd.memset(a_sb[:, 0:2, :], 0.0)
    nc.gpsimd.memset(a_sb[:, H + 2:H + 4, :], 0.0)
    nc.gpsimd.memset(a_sb[:, 2:H + 2, 0:2], 0.0)
    nc.gpsimd.memset(a_sb[:, 2:H + 2, W + 2:W + 4], 0.0)

    # conv group assignment: 18 matmuls round-robin over 4 col groups
    pairs = [(di, dj, ch) for di in range(3) for dj in range(3)
             for ch in range(NCH)]
    grp = {p: i % 4 for i, p in enumerate(pairs)}
    gfirst, glast = {}, {}
    for p in pairs:
        g = grp[p]
        if g not in gfirst:
            gfirst[g] = p
        glast[g] = p

    HHALF = H // 2

    def conv_strip(s):
        xb = load_x_strip(s)
        r0 = s * RSTRIP
        for q in range(NSLICE):
            h0 = r0 + 4 * q
            f4 = f4pp.tile([128, 4, W], F32, tag="f4p")
            for (di, dj, ch) in pairs:
                g = grp[(di, dj, ch)]
                t32 = ((di * 3 + dj) * 2 + ch) * 32
                nc.tensor.matmul(
                    f4[32 * g:32 * g + 32, :, :],
                    wc[:, t32:t32 + 32],
                    xb[:, ch, 4 * q + di:4 * q + di + 4, 1 + dj:1 + dj + W],
                    start=(gfirst[g] == (di, dj, ch)),
                    stop=(glast[g] == (di, dj, ch)),
                    tile_position=(0, 32 * g),
                    skip_group_check=True,
                )
            f4s = f4sp.tile([128, 4, W], F32R, tag="f4s")
            nc.scalar.copy(f4s[:], f4[:])
            apt = app.tile([128, 4, W], F32, tag="ap")
            nc.tensor.matmul(apt[:], r_r[:], f4s[:], start=True, stop=True)
            nc.scalar.activation(a_sb[:, h0 + 2:h0 + 6, 2:W + 2],
                                 apt[:], AFT.Identity, bias=bias_t[:])

    # window-build issue paths over the two HWDGE rings (SWDGE would pay a
    # Q7 descriptor storm for these strided row-wise copies)
    B_ENG = [lambda: nc.sync, lambda: nc.scalar, lambda: nc.sync,
             lambda: nc.scalar]

    def build_b_half(half):
        """Window copies as FLAT contiguous-run DMAs: copy HHALF full padded
        rows of a_sb starting at flat offset (r0+ki)*PW + kj. Row r of the
        dst then holds a_sb[., r0+ki+r, kj:] with the tail of each 132-wide
        row wrapping into the next source row — the wrap garbage lands in
        dst cols >= 128 which no consumer reads. One descriptor per
        partition instead of one per 256-byte row."""
        r0 = half * HHALF
        bts = []
        for j, (i0, i1, j0, j1) in enumerate(RECTS):
            bt = bpool.tile([128, HHALF, PW], BF16, tag=f"b{j}")
            eng = B_ENG[j]()
            idx = 0
            for ki in range(i0, i1):
                for kj in range(j0, j1):
                    st = (r0 + ki) * PW + kj
                    src = a_flat[0:CC, st:st + HHALF * PW].rearrange(
                        "p (h w) -> p h w", w=PW)
                    eng.dma_start(bt[CC * idx:CC * (idx + 1), :, :], src)
                    idx += 1
            bts.append(bt)
        return bts

    def autocorr_strip(s, bts):
        r0 = s * RSTRIP
        rb = r0 % HHALF
        ot = opool.tile([COUT, RSTRIP, W], F32, tag="osb")
        for q in range(NSLICE):
            h0 = r0 + 4 * q
            op = opp.tile([COUT, 4, W], F32, tag="op")
            for j in range(4):
                pj = RECT_P[j]
                gt = gpool.tile([128, 4, W], BF16, tag="g")
                eng = nc.gpsimd if j == 3 else nc.vector
                eng.tensor_mul(
                    gt[0:pj, :, :],
                    a_sb[0:pj, h0 + 2:h0 + 6, 2:W + 2],
                    bts[j][0:pj, rb + 4 * q:rb + 4 * q + 4, 0:W])
                nc.tensor.matmul(op[:], wr[0:pj, j * COUT:(j + 1) * COUT],
                                 gt[0:pj, :, :],
                                 start=(j == 0), stop=(j == 3))
            if q % 2 == 0:
                nc.vector.tensor_copy(ot[:, 4 * q:4 * q + 4, :], op[:])
            else:
                nc.scalar.copy(ot[:, 4 * q:4 * q + 4, :], op[:])
        # contiguous per-partition runs on both sides -> cheap on SWDGE
        nc.gpsimd.dma_start(out[:, r0:r0 + RSTRIP, :], ot[:])

    if phase == "bonly":
        nc.gpsimd.memset(a_sb[:], 0.0)
        build_b_half(0)
        build_b_half(1)
        return

    for s in range(5):
        conv_strip(s)
    if phase == "conv":
        return
    bts0 = build_b_half(0)
    for s in range(5, NSTRIP):
        conv_strip(s)
    if phase == "convb":
        build_b_half(1)
        return
    for s in range(4):
        autocorr_strip(s, bts0)
    bts1 = build_b_half(1)
    for s in range(4, NSTRIP):
        autocorr_strip(s, bts1)


_NC = None


def _get_nc():
    global _NC
    if _NC is None:
        _NC = _build_nc()
    return _NC


def _shard_inputs(x, W_ext, b_ext, W_reg):
    xb = np.asarray(x, dtype=np.float32).astype(ml_dtypes.bfloat16)
    xpad = np.zeros((B, CIN, H + 2, W + 4), dtype=ml_dtypes.bfloat16)
    xpad[:, :, 1:H + 1, 2:W + 2] = xb
    web = np.asarray(W_ext, dtype=np.float32).astype(ml_dtypes.bfloat16)
    wrb = np.asarray(W_reg, dtype=np.float32).astype(ml_dtypes.bfloat16)
    bf = np.ascontiguousarray(np.asarray(b_ext, dtype=np.float32))
    return [
        {"x": np.ascontiguousarray(xpad[c]), "W_ext": web, "b_ext": bf,
         "W_reg": wrb}
        for c in range(B)
    ]


def kernel(x, W_ext, b_ext, W_reg):
    nc = _get_nc()
    in_maps = _shard_inputs(x, W_ext, b_ext, W_reg)
    res = run_bass_kernel_spmd(nc, in_maps, core_ids=list(range(B)))
    return np.stack([res.results[c]["out"] for c in range(B)], axis=0)


if __name__ == "__main__":
    rng = np.random.default_rng(0)
    inputs = {
        "x": rng.standard_normal((B, CIN, H, W), dtype=np.float32),
        "W_ext": (rng.standard_normal((CC, CIN, 3, 3)) * 0.05).astype(np.float32),
        "b_ext": (rng.standard_normal((CC,)) * 0.05).astype(np.float32),
        "W_reg": (rng.standard_normal((COUT, CC, 5, 5)) * 0.05).astype(np.float32),
    }
    y = kernel(**inputs)
    print("out", y.shape, y.dtype, float(np.abs(y).max()))
